# revision 8
# baseline (speedup 1.0000x reference)
"""Trainium2 Bass kernel for nn_CNP_MLP_Mean (CNP encoder/decoder, mean pool).

Strategy (v2)
-------------
Pure data parallel: B=32 samples, 4 per core over 8 NeuronCores.  All
activations feature-major ([feature, token] with features on SBUF
partitions); biases fold into per-partition scalars; W2 is fused into
W3/W5 on the host so the device runs three matmul layers (L1: 64->256,
L2: 256->128 fused with the decoder first layer, L6: 128->2).

The per-tile psum->sbuf "crossing" (relu/copy of the hidden layer and the
decoder layer) is the hard floor: only ACT and DVE can read PSUM.  Per
512-token tile the hidden crossing is routed one of three ways to balance
all four engines:
  A: pos-add via PE identity matmul, ACT relu  (PSUM -> SBUF)
  B: DVE tensor_tensor add(+pos) (crossing), Pool relu in SBUF
  K: ACT copy (crossing), Pool add(+pos), Pool relu (both SBUF)
and the decoder relu (d1) splits between ACT (bias via activation) and
DVE (tensor_scalar add+max).

The context branch is batched across all 4 samples into single wide ops
(one bias vector b5t[128, 4] for the whole core).  Its ~8.5us latency is
hidden by lagging each sample's L2/d1/L6 chain a few tiles behind its
hidden-crossing stage (psb PSUM recycling limits the L2->d1 distance, so
the lag is between crossing and L2).

DMA: everything rides 128 partitions (the cost model charges
bytes-per-partition), weights+biases travel as one byte-packed tensor
(f32 biases bitcast into bf16 columns), and the big streams are spread
over the four engine queues so transfers overlap.
"""

import numpy as np
import ml_dtypes
import os as _os
from contextlib import ExitStack

import concourse.bass as bass
import concourse.bacc as bacc
import concourse.mybir as mybir
import concourse.tile as tile
from concourse.bass import ts
from concourse.bass_utils import run_bass_kernel_spmd

B, L, U, HX, XD, RD, C = 32, 4096, 64, 256, 128, 128, 256
STD = 0.1
NCORES = 8
BLOC = B // NCORES
TOK = 512
NT = L // TOK          # 8 tiles per sample
NG = L // 128          # 32 groups per sample
NTILE = BLOC * NT      # 32 tile-slots per core

F32 = mybir.dt.float32
BF16 = mybir.dt.bfloat16
AF = mybir.ActivationFunctionType
OP = mybir.AluOpType
BF = ml_dtypes.bfloat16

# --- knobs ----------------------------------------------------------------
# crossing path per tile-slot (b major, t_order minor): A | B | K
PATHS = _os.environ.get(
    "PATHS", "KBBKBBBA" "KBBKBBBA" "KBBKBBBA" "KBBKBBBA")
# d1 engine per tile-slot: A (ACT) | V (DVE)
D1ENG = _os.environ.get(
    "D1ENG", "AAVAAVAA" "AAVAAVAA" "AAVAAVAA" "AAVAAVAA")
# lag (in tile-slots) between crossing(b,t) and L2/d1/L6(b,t), per sample
OFFS = [int(x) for x in _os.environ.get("OFFS", "8,4,2,2").split(",")]
# tile processing order within a sample (ft chunk A covers tiles 0,1,4,5)
T_ORDER = [int(x) for x in _os.environ.get("TORD", "0,1,4,5,2,3,6,7").split(",")]

# wpack column layout (bf16 cols)
WP_W25 = 0        # [128, 2*128] w25k halves
WP_W23 = 256      # [128, 2*128]
WP_W45 = 512      # [128, 128]
WP_W6 = 640       # [128, 2]
WP_W3Y = 642      # row 0 only, [1, 128]
WP_BIAS = 772     # 4 f32 per partition = 8 bf16 cols (byte offset 1544, /4 ok)
WP_COLS = 780


def _build_nc():
    nc = bacc.Bacc("TRN2")

    # ---- DRAM I/O ----
    # per-sample features folded onto 128 partitions:
    #   partition p<64:  u=p, tokens 0:2047 ; p>=64: u=p-64, tokens 2048:4095
    ftd = nc.dram_tensor("ft", [BLOC, 128, L // 2], BF16, kind="ExternalInput")
    pbi = nc.dram_tensor("posb1i", [128, NT * 2 * TOK], BF16, kind="ExternalInput")
    fcd = nc.dram_tensor("fctx", [128, BLOC * C // 2], BF16, kind="ExternalInput")
    pcd = nc.dram_tensor("posctx", [128, 2 * BLOC * C], BF16, kind="ExternalInput")
    ycd = nc.dram_tensor("yctx", [1, BLOC * C], BF16, kind="ExternalInput")
    wpd = nc.dram_tensor("wpack", [128, WP_COLS], BF16, kind="ExternalInput")
    w1d = nc.dram_tensor("w1", [128, 256], BF16, kind="ExternalInput")  # W1 duplicated on both partition halves

    yvd = nc.dram_tensor("yvbuf", [128, 2 * BLOC * NG], F32, kind="ExternalOutput")

    with tile.TileContext(nc) as tc, ExitStack() as ctx:
        const = ctx.enter_context(tc.tile_pool(name="const", bufs=1))
        hpool = ctx.enter_context(tc.tile_pool(name="h", bufs=int(_os.environ.get("HB", "12"))))
        dpool = ctx.enter_context(tc.tile_pool(name="d", bufs=int(_os.environ.get("DB", "4"))))
        cpool = ctx.enter_context(tc.tile_pool(name="c", bufs=1))
        psA = ctx.enter_context(tc.tile_pool(name="psA", bufs=2, space="PSUM"))
        psB = ctx.enter_context(tc.tile_pool(name="psB", bufs=2, space="PSUM"))
        psO = ctx.enter_context(tc.tile_pool(name="psO", bufs=2, space="PSUM"))

        # --- PE warm-up: start the p-state ramp clock immediately ---------
        wz = const.tile([64, 16], BF16, name="wz")
        nc.gpsimd.memset(wz[:], 0.0)
        pwu = psB.tile([128, TOK], F32, tag="psb")
        nc.tensor.matmul(pwu[:16, :16], lhsT=wz[:, :16], rhs=wz[:, :16],
                         start=True, stop=True)

        # --- identity for PE pos-add (path A + ctx) -----------------------
        ident = const.tile([128, 128], BF16)
        from concourse.masks import make_identity
        make_identity(nc, ident[:])

        # --- DMA issue plan ----------------------------------------------
        # Pool: w1, pbi-c0, wpack, pbi-c2   (each ~500ns engine hold)
        # DVE : fctx, posctx, yctx          (needed by ctx early)
        # ACT : pbi-c1, pbi-c3
        # SP  : ft chunks (b0 in halves, then b1..b3)
        w1 = const.tile([128, 256], BF16, name="w1")
        nc.gpsimd.dma_start(w1[:], w1d[:])
        posb1i = const.tile([128, NT * 2 * TOK], BF16, name="posb1i")
        PQ = NT * 2 * TOK // 4  # 2048-col chunk = pos for tiles {2c, 2c+1}
        nc.gpsimd.dma_start(posb1i[:, ts(0, PQ)], pbi[:, ts(0, PQ)])
        wpack = const.tile([128, WP_COLS], BF16, name="wpack")
        nc.gpsimd.dma_start(wpack[:], wpd[:])
        nc.gpsimd.dma_start(posb1i[:, ts(2, PQ)], pbi[:, ts(2, PQ)])

        fc = const.tile([128, BLOC * C // 2], BF16, name="fc")
        nc.scalar.dma_start(fc[:], fcd[:])
        pctx = const.tile([128, 2 * BLOC * C], BF16, name="pctx")
        nc.scalar.dma_start(pctx[:], pcd[:])
        yct = const.tile([1, BLOC * C], BF16, name="yct")
        nc.scalar.dma_start(yct[:], ycd[:])

        ft_s = [const.tile([128, L // 2], BF16, name=f"ft_{b}") for b in range(BLOC)]
        HAL = L // 4
        nc.sync.dma_start(ft_s[0][:, :HAL], ftd[0][:, :HAL])
        nc.sync.dma_start(ft_s[0][:, HAL:], ftd[0][:, HAL:])
        nc.sync.dma_start(posb1i[:, ts(1, PQ)], pbi[:, ts(1, PQ)])
        nc.sync.dma_start(posb1i[:, ts(3, PQ)], pbi[:, ts(3, PQ)])
        for b in range(1, BLOC):
            nc.sync.dma_start(ft_s[b][:], ftd[b])

        # weight slices
        w25 = [wpack[:, WP_W25 + 128 * k:WP_W25 + 128 * (k + 1)] for k in (0, 1)]
        w23 = [wpack[:, WP_W23 + 128 * k:WP_W23 + 128 * (k + 1)] for k in (0, 1)]
        w45 = wpack[:, WP_W45:WP_W45 + 128]
        w6 = wpack[:, WP_W6:WP_W6 + 2]
        w3y = wpack[0:1, WP_W3Y:WP_W3Y + 128]
        bias4 = wpack[:, WP_BIAS:WP_BIAS + 8].bitcast(F32)  # [128, 4]
        b3a = bias4[:, 0:1]
        b5a = bias4[:, 1:2]
        b6y = bias4[:, 2:3]
        b6v = bias4[:, 3:4]

        def ft_tile(b, t):
            q, c = divmod(t, NT // 2)
            return ft_s[b][64 * q:64 * (q + 1), ts(c, TOK)]

        # ================= context branch (all samples batched) ==========
        # Emitted in 4 stages woven between early main-loop slots so no
        # engine queue head-blocks on the ctx chain's latency.
        CW = BLOC * C  # 1024 context tokens
        ctx_state = {}

        def ctx_stage_a():  # PE: hc matmuls + identity pos
            hc0 = psA.tile([128, CW], F32, tag="psa", name="hc0")
            hc1 = psA.tile([128, CW], F32, tag="psa", name="hc1")
            for q in (0, 1):  # fctx folded: partitions 64q.., cols [512q..]
                fcq = fc[64 * q:64 * (q + 1), :]
                for h, hc in ((0, hc0), (1, hc1)):
                    nc.tensor.matmul(hc[:, ts(q, CW // 2)],
                                     lhsT=w1[64 * q:64 * (q + 1), ts(h, 128)],
                                     rhs=fcq,
                                     start=True, stop=False)
            for h, hc in ((0, hc0), (1, hc1)):
                for c2 in range(CW // TOK):
                    nc.tensor.matmul(
                        hc[:, ts(c2, TOK)], lhsT=ident[:],
                        rhs=pctx[:, h * CW + c2 * TOK: h * CW + (c2 + 1) * TOK],
                        start=False, stop=True)
            ctx_state["hc"] = (hc0, hc1)

        def ctx_stage_b():  # ACT + DVE: hcb relus
            hc0, hc1 = ctx_state.pop("hc")
            hcb = cpool.tile([128, 2 * CW], BF16, tag="hcb")
            nc.scalar.activation(hcb[:, :CW], hc0[:], AF.Relu)
            nc.vector.tensor_scalar_max(hcb[:, CW:], hc1[:], 0.0)
            ctx_state["hcb"] = hcb

        def ctx_stage_c():  # PE: pr1; ACT: r1
            hcb = ctx_state.pop("hcb")
            pr1 = psA.tile([128, CW], F32, tag="psa", name="pr1")
            for c2 in range(CW // TOK):
                sl = ts(c2, TOK)
                nc.tensor.matmul(pr1[:, sl], lhsT=w23[0],
                                 rhs=hcb[:, c2 * TOK:(c2 + 1) * TOK],
                                 start=True, stop=False)
                nc.tensor.matmul(pr1[:, sl], lhsT=w23[1],
                                 rhs=hcb[:, CW + c2 * TOK:CW + (c2 + 1) * TOK],
                                 start=False, stop=False)
                nc.tensor.matmul(pr1[:, sl], lhsT=w3y, rhs=yct[:, sl],
                                 start=False, stop=True)
            r1 = cpool.tile([128, CW], BF16, tag="r1")
            nc.scalar.activation(r1[:], pr1[:], AF.Relu, bias=b3a)
            ctx_state["r1"] = r1

        def ctx_stage_d():  # DVE: mean; PE: w45; ACT: b5t
            r1 = ctx_state.pop("r1")
            rs = cpool.tile([128, BLOC], F32, tag="rs")
            nc.vector.tensor_reduce(
                rs[:], r1[:].rearrange("p (b c) -> p b c", b=BLOC),
                mybir.AxisListType.X, OP.add)
            rm = cpool.tile([128, BLOC], BF16, tag="rm")
            nc.vector.tensor_scalar_mul(rm[:], rs[:], 1.0 / C)
            pb5 = psO.tile([128, NG, 2], F32, tag="pso", name="pb5")
            nc.tensor.matmul(pb5[:, 0:2, :], lhsT=w45, rhs=rm[:],
                             start=True, stop=True)
            b5t = const.tile([128, BLOC], F32, name="b5t")
            nc.scalar.activation(b5t[:], pb5[:, 0:2, :], AF.Identity, bias=b5a)
            ctx_state["b5t"] = b5t

        CTX_AT = [int(x) for x in _os.environ.get("CTXAT", "0,1,2,4").split(",")]
        ctx_stages = {CTX_AT[0]: ctx_stage_a, CTX_AT[1]: ctx_stage_b,
                      CTX_AT[2]: ctx_stage_c, CTX_AT[3]: ctx_stage_d}

        # ================= main pipeline =================================
        yv = const.tile([128, 2 * BLOC * NG], F32, name="yv")

        pend = []      # tiles awaiting L2/d1/L6: (b, t, hb_tile)
        pend_l6 = []   # one-tile lag for L6 so PE never waits on d1
        pso_s = {}     # per-sample L6 psum
        drained = {b: 0 for b in range(BLOC)}

        def emit_l6(b, t, dt_):
            if b not in pso_s:
                pso_s[b] = psO.tile([128, NG, 2], F32, tag="pso", name=f"pso_{b}")
            pso = pso_s[b]
            for g in range(TOK // 128):
                nc.tensor.matmul(pso[:, t * (TOK // 128) + g, :],
                                 lhsT=dt_[:, ts(g, 128)], rhs=w6[:],
                                 start=True, stop=True)

        def emit_sample_out(b):
            pso = pso_s.pop(b)
            nc.vector.tensor_scalar_add(yv[:, ts(b, NG)], pso[:, :, 0], b6y)
            nc.scalar.activation(yv[:, BLOC * NG + b * NG:BLOC * NG + (b + 1) * NG],
                                 pso[:, :, 1], AF.Copy)

        def emit_back(n):
            """Emit L2+d1 (+lagged L6) for up to n pending tiles."""
            for _ in range(min(n, len(pend))):
                b, t, hb = pend.pop(0)
                s = b * NT + t
                psb_ = psB.tile([128, TOK], F32, tag="psb")
                nc.tensor.matmul(psb_[:], lhsT=w25[0], rhs=hb[:, :TOK],
                                 start=True, stop=False)
                nc.tensor.matmul(psb_[:], lhsT=w25[1], rhs=hb[:, TOK:],
                                 start=False, stop=True)
                dt_ = dpool.tile([128, TOK], BF16)
                if D1ENG[s] == "A":
                    nc.scalar.activation(dt_[:], psb_[:], AF.Relu,
                                         bias=b5t_ref[0][:, b:b + 1])
                else:
                    nc.vector.tensor_scalar(dt_[:], psb_[:],
                                            b5t_ref[0][:, b:b + 1],
                                            0.0, OP.add, OP.max)
                pend_l6.append((b, t, dt_))
                if len(pend_l6) > 1:
                    bb, tt, dd = pend_l6.pop(0)
                    emit_l6(bb, tt, dd)
                    drained[bb] += 1
                    if drained[bb] == NT:
                        emit_sample_out(bb)

        b5t_ref = [None]
        slot = 0
        for b in range(BLOC):
            off = OFFS[b] if b < len(OFFS) else 2
            for t in T_ORDER:
                if slot in ctx_stages:
                    ctx_stages[slot]()
                    if "b5t" in ctx_state:
                        b5t_ref[0] = ctx_state["b5t"]
                s = b * NT + t
                path = PATHS[s]
                psa = psA.tile([128, 2 * TOK], F32, tag="psa")
                pe_pos = path == "A"
                q = t // (NT // 2)
                for half in (0, 1):
                    nc.tensor.matmul(psa[:, ts(half, TOK)],
                                     lhsT=w1[64 * q:64 * (q + 1), ts(half, 128)],
                                     rhs=ft_tile(b, t),
                                     start=True, stop=not pe_pos)
                if pe_pos:
                    for half in (0, 1):
                        nc.tensor.matmul(
                            psa[:, ts(half, TOK)], lhsT=ident[:],
                            rhs=posb1i[:, 2 * TOK * t + half * TOK:
                                       2 * TOK * t + (half + 1) * TOK],
                            start=False, stop=True)
                hb = hpool.tile([128, 2 * TOK], BF16)
                if path == "A":
                    nc.scalar.activation(hb[:], psa[:], AF.Relu)
                elif path == "B":
                    nc.vector.tensor_tensor(hb[:], psa[:],
                                            posb1i[:, ts(t, 2 * TOK)], OP.add)
                    nc.gpsimd.tensor_relu(hb[:], hb[:])
                else:  # K
                    nc.scalar.activation(hb[:], psa[:], AF.Copy)
                    nc.gpsimd.tensor_tensor(hb[:], hb[:],
                                            posb1i[:, ts(t, 2 * TOK)], OP.add)
                    nc.gpsimd.tensor_relu(hb[:], hb[:])
                pend.append((b, t, hb))
                slot += 1
                if len(pend) > off:
                    emit_back(len(pend) - off)
        # drain
        emit_back(len(pend))
        while pend_l6:
            bb, tt, dd = pend_l6.pop(0)
            emit_l6(bb, tt, dd)
            drained[bb] += 1
            if drained[bb] == NT:
                emit_sample_out(bb)

        # softplus tail: v = 0.1 + 0.9*ln(1+exp(x + b6v))
        vsl = yv[:, BLOC * NG:]
        nc.scalar.activation(vsl, vsl, AF.Exp, bias=b6v)
        nc.scalar.activation(vsl, vsl, AF.Ln, bias=1.0)
        nc.vector.tensor_scalar(vsl, vsl, 0.9, 0.1, OP.mult, OP.add)
        nc.sync.dma_start(yvd[:], yv[:])

    nc.compile()
    return nc


_NC = None


def _get_nc():
    global _NC
    if _NC is None:
        _NC = _build_nc()
    return _NC


def _host_prep(features, indexes, context, lens, noise,
               W1, b1, W2, b2, W3, b3, W4, b4, W5, b5, W6, b6):
    features = np.asarray(features, np.float32)
    indexes = np.asarray(indexes, np.int64)
    context = np.asarray(context, np.float32)
    noise = np.asarray(noise, np.float32)
    W1 = np.asarray(W1, np.float32); b1 = np.asarray(b1, np.float32)
    W2 = np.asarray(W2, np.float32); b2 = np.asarray(b2, np.float32)
    W3 = np.asarray(W3, np.float32); b3 = np.asarray(b3, np.float32)
    W4 = np.asarray(W4, np.float32); b4 = np.asarray(b4, np.float32)
    W5 = np.asarray(W5, np.float32); b5 = np.asarray(b5, np.float32)
    W6 = np.asarray(W6, np.float32); b6 = np.asarray(b6, np.float32)

    k = np.arange(L, dtype=np.float32)[:, None]
    i = np.arange(HX // 2, dtype=np.float32)[None, :]
    ang = k / np.power(np.float32(10000.0), 2.0 * i / HX)
    pos = np.zeros((L, HX), np.float32)
    pos[:, 0::2] = np.sin(ang)
    pos[:, 1::2] = np.cos(ang)
    posb1 = pos + b1
    posb1_fm = posb1.T.astype(BF)  # [HX, L]
    pbi = np.stack([posb1_fm[:128].reshape(128, NT, TOK),
                    posb1_fm[128:].reshape(128, NT, TOK)], axis=2)
    pbi = np.ascontiguousarray(pbi.reshape(128, NT * 2 * TOK))

    yc = context + STD * noise

    w25k = (W2.astype(np.float64) @ W5[:XD].astype(np.float64)).astype(np.float32)
    w23k = (W2.astype(np.float64) @ W3[:XD].astype(np.float64)).astype(np.float32)
    w45k = (W4.astype(np.float64) @ W5[XD:].astype(np.float64)).astype(np.float32)
    b3a = (b3 + b2 @ W3[:XD]).astype(np.float32)
    b5a = (b5 + b2 @ W5[:XD] + b4 @ W5[XD:]).astype(np.float32)

    wpack = np.zeros((128, WP_COLS), BF)
    wpack[:, WP_W25:WP_W25 + 128] = w25k[:128].astype(BF)
    wpack[:, WP_W25 + 128:WP_W25 + 256] = w25k[128:].astype(BF)
    wpack[:, WP_W23:WP_W23 + 128] = w23k[:128].astype(BF)
    wpack[:, WP_W23 + 128:WP_W23 + 256] = w23k[128:].astype(BF)
    wpack[:, WP_W45:WP_W45 + 128] = w45k.astype(BF)
    wpack[:, WP_W6:WP_W6 + 2] = W6.astype(BF)
    wpack[0, WP_W3Y:WP_W3Y + 128] = W3[XD:XD + 1].astype(BF)[0]
    bias4 = np.zeros((128, 4), np.float32)
    bias4[:, 0] = b3a
    bias4[:, 1] = b5a
    bias4[:, 2] = b6[0]
    bias4[:, 3] = b6[1]
    wpack[:, WP_BIAS:WP_BIAS + 8] = bias4.view(np.uint16).view(BF)

    common = {
        "posb1i": pbi,
        "wpack": wpack,
        "w1": np.ascontiguousarray(np.vstack([W1.astype(BF), W1.astype(BF)])),
    }

    in_maps = []
    for cix in range(NCORES):
        sl = slice(cix * BLOC, (cix + 1) * BLOC)
        f_c = features[sl]
        idx_c = indexes[sl]
        # folded ft: [128, L/2]
        ft = np.empty((BLOC, 128, L // 2), BF)
        for j in range(BLOC):
            t64 = f_c[j].T.astype(BF)  # [64, L]
            ft[j][:64] = t64[:, :L // 2]
            ft[j][64:] = t64[:, L // 2:]
        # gathered ctx features folded: [128, BLOC*C/2]
        fcat = np.concatenate(
            [f_c[j][idx_c[j]].T.astype(BF) for j in range(BLOC)], axis=1)  # [64, 1024]
        fctx = np.empty((128, BLOC * C // 2), BF)
        fctx[:64] = fcat[:, :BLOC * C // 2]
        fctx[64:] = fcat[:, BLOC * C // 2:]
        # pos at ctx points: [256, C] per sample -> a|b halves concat
        pctx = np.concatenate(
            [posb1_fm[:, idx_c[j]] for j in range(BLOC)], axis=1)  # [256, 1024]
        posctx = np.concatenate([pctx[:128], pctx[128:]], axis=1)  # [128, 2048]
        m = dict(common)
        m["ft"] = np.ascontiguousarray(ft)
        m["fctx"] = np.ascontiguousarray(fctx)
        m["posctx"] = np.ascontiguousarray(posctx.astype(BF))
        m["yctx"] = np.ascontiguousarray(yc[sl].reshape(1, BLOC * C).astype(BF))
        in_maps.append(m)
    return in_maps


def _assemble(results):
    y = np.empty((B, L), np.float32)
    v = np.empty((B, L), np.float32)
    for c, r in enumerate(results):
        yv = np.asarray(r["yvbuf"], np.float32)
        yb = yv[:, :BLOC * NG].reshape(128, BLOC, NG)
        vb = yv[:, BLOC * NG:].reshape(128, BLOC, NG)
        for j in range(BLOC):
            y[c * BLOC + j] = yb[:, j, :].T.reshape(L)
            v[c * BLOC + j] = vb[:, j, :].T.reshape(L)
    return y, v


def kernel(**inputs):
    nc = _get_nc()
    in_maps = _host_prep(**inputs)
    res = run_bass_kernel_spmd(nc, in_maps, list(range(NCORES)))
    return _assemble(res.results)


# ---------------------------------------------------------------------------
# Timing utilities (no NTFF profiler hook under this axon site).

_RUNNER = None


def _make_runner(nc):
    import jax
    from jax.sharding import Mesh, PartitionSpec, NamedSharding
    from jax.experimental.shard_map import shard_map
    import concourse.mybir as _mb
    from concourse import bass2jax

    bass2jax.install_neuronx_cc_hook()
    partition_name = nc.partition_id_tensor.name if nc.partition_id_tensor else None
    in_names, out_names, out_avals, zero_shapes = [], [], [], []
    for alloc in nc.m.functions[0].allocations:
        if not isinstance(alloc, _mb.MemoryLocationSet):
            continue
        name = alloc.memorylocations[0].name
        if alloc.kind == "ExternalInput":
            if name != partition_name:
                in_names.append(name)
        elif alloc.kind == "ExternalOutput":
            out_names.append(name)
            shape = tuple(alloc.tensor_shape)
            dtype = _mb.dt.np(alloc.dtype)
            out_avals.append(jax.core.ShapedArray(shape, dtype))
            zero_shapes.append((shape, dtype))
    n_params = len(in_names)
    donate = tuple(range(n_params, n_params + len(out_names)))
    bind_names = tuple(in_names + out_names
                       + ([partition_name] if partition_name else []))

    def _body(*args):
        operands = list(args)
        if partition_name is not None:
            operands.append(bass2jax.partition_id_tensor())
        outs = bass2jax._bass_exec_p.bind(
            *operands,
            out_avals=tuple(out_avals),
            in_names=bind_names,
            out_names=tuple(out_names),
            lowering_input_output_aliases=(),
            sim_require_finite=True,
            sim_require_nnan=True,
            nc=nc,
        )
        return tuple(outs)

    devices = jax.devices()[:NCORES]
    mesh = Mesh(np.asarray(devices), ("core",))
    spec = PartitionSpec("core")
    sharded = jax.jit(
        shard_map(_body, mesh=mesh,
                  in_specs=(spec,) * (n_params + len(out_names)),
                  out_specs=(spec,) * len(out_names), check_rep=False),
        donate_argnums=donate, keep_unused=True)
    sh = NamedSharding(mesh, spec)

    class Runner:
        def put(self, in_maps):
            arrs = []
            for name in in_names:
                cat = np.concatenate([np.asarray(m[name]) for m in in_maps], axis=0)
                arrs.append(jax.device_put(cat, sh))
            return arrs

        def zeros(self):
            return [jax.device_put(
                np.zeros((NCORES * s[0], *s[1:]), d), sh) for s, d in zero_shapes]

        def run(self, staged, zeros=None):
            return sharded(*staged, *(zeros if zeros is not None else self.zeros()))

        def results(self, outs):
            return [
                {name: np.asarray(outs[i]).reshape(NCORES, *out_avals[i].shape)[c]
                 for i, name in enumerate(out_names)}
                for c in range(NCORES)]

    return Runner()


def get_runner():
    global _RUNNER
    if _RUNNER is None:
        _RUNNER = _make_runner(_get_nc())
    return _RUNNER


def bench(inputs, iters=30):
    import time as _t
    import jax
    r = get_runner()
    staged = r.put(_host_prep(**inputs))
    outs = r.run(staged)
    jax.block_until_ready(outs)
    zpool = [r.zeros() for _ in range(iters)]
    for z in zpool:
        jax.block_until_ready(z)
    times = []
    for i in range(iters):
        t0 = _t.perf_counter()
        outs = r.run(staged, zpool[i])
        jax.block_until_ready(outs)
        times.append(_t.perf_counter() - t0)
    y, v = _assemble(r.results(outs))
    return (y, v), times


def sim_time():
    """Cost-model simulated kernel duration in ns (core 0)."""
    from concourse import bass_interp
    import jax
    import reference  # noqa - only available in the dev workspace
    with jax.default_device(jax.devices("cpu")[0]):
        inputs = {k: np.asarray(v) for k, v in reference.setup_inputs().items()}
    nc = _get_nc()
    in_maps = _host_prep(**inputs)
    sim = bass_interp.CoreSim(
        nc, trace=True, scheduler=bass_interp.DefaultScheduler(respect_deps=True))
    for name, val in in_maps[0].items():
        sim.tensor(name)[:] = val
    sim.simulate()
    return sim._sim_state.time
